# revision 28
# baseline (speedup 1.0000x reference)
"""Distributed Trainium2 Bass kernel for the phasor attention problem
(nn_Attention_17798344475248).

Sharding: 8 cores = 2 batches x 4 head-groups (2 heads each). Each core
computes its batch's Q/K/V projections for its 2 heads, phasor attention,
and a partial final-dense output; partials are summed with 4 pipelined
4-rank ReduceScatters per batch group; each core finishes atan2 on its
4x64-row slices of the output.

Math notes (vs reference.py):
- phasor_encode(phasor_act(z)) == z/|z|  -> normalize instead of atan2+cos/sin
- softmax max-subtract and sum-normalization cancel in the final angle
  (positive per-row scale), so softmax reduces to exp(s/d)
- complex bias (real) folds into the ACT Square/rescale passes for Q/K
  (per-partition bias) and stays a K=1 outer-product matmul for V / final
- all matmul operands are fp16 (10-bit mantissa, 1 PE cycle/row, fast
  weight load) with f32 PSUM accumulation
- phasor encodes (the only Sin-set ACT work) run in one pipelined phase
  at the start and stay resident in SBUF as fp16
"""
import sys

sys.path.insert(0, "/opt/trn_rl_repo")

import numpy as np

import concourse.bass as bass
import concourse.tile as tile
from concourse import bacc, mybir
from concourse.bass_utils import run_bass_kernel_spmd
from concourse.masks import make_identity

F32 = mybir.dt.float32
F16 = mybir.dt.float16
AF = mybir.ActivationFunctionType
ALU = mybir.AluOpType
PI = float(np.pi)

B, T, D, H = 2, 1024, 512, 8
P = 128
DS = D // P          # 4 partition-slices of the model dim
CH = 512             # chunk width along t (both q and kv passes)
NCH = T // CH        # 2 chunks
N_CORES = 8
HPC = 2              # heads per core


def build(debug=False):
    nc = bacc.Bacc("TRN2", target_bir_lowering=False, debug=False,
                   num_devices=N_CORES)
    cpi2 = nc.alloc_sbuf_tensor("const-f32-pi2", [P, 1], F32)
    nc.gpsimd.memset(cpi2.ap(), PI / 2)
    nc.const_aps.aps[(F32, PI / 2)] = cpi2.ap()
    nc.all_engine_barrier()

    # ---- I/O ----
    QUERY = nc.dram_tensor("query", [T, D], F32, kind="ExternalInput")
    KEYVALUE = nc.dram_tensor("keyvalue", [T, D], F32, kind="ExternalInput")
    WQ = nc.dram_tensor("wq", [HPC, D, D], F32, kind="ExternalInput")
    WK = nc.dram_tensor("wk", [HPC, D, D], F32, kind="ExternalInput")
    WV = nc.dram_tensor("wv", [HPC, D, D], F32, kind="ExternalInput")
    BQ = nc.dram_tensor("bq", [HPC, D], F32, kind="ExternalInput")
    BK = nc.dram_tensor("bk", [HPC, D], F32, kind="ExternalInput")
    BV = nc.dram_tensor("bv", [HPC, D], F32, kind="ExternalInput")
    WO = nc.dram_tensor("wo", [HPC * D, D], F32, kind="ExternalInput")
    BO = nc.dram_tensor("bo", [D], F32, kind="ExternalInput")
    OUT = nc.dram_tensor("out", [T // 4, D], F32, kind="ExternalOutput")

    with tile.TileContext(nc) as tc:
        import contextlib
        with contextlib.ExitStack() as ctx:
            pools = {}
            for name, bufs, space in [
                ("persist", 1, "SBUF"),
                ("raw", 4, "SBUF"),       # 2KB x4 raw input tiles
                ("nt", 6, "SBUF"),        # 2KB x6 norm/atan2 temps
                ("w", 1, "SBUF"),         # 8KB f32 weight staging
                ("w16", 4, "SBUF"),       # 2KB x4 fp16 weights (per head)
                ("brow", 6, "SBUF"),      # small bias rows/cols
                ("enc", 1, "SBUF"),       # 32KB: q/kv cos/sin fp16 (4 tags)
                ("kt", 2, "SBUF"),        # 16KB: K^T fp16 (re+im)
                ("v", 2, "SBUF"),         # 16KB: V fp16 (re+im)
                ("qt", 4, "SBUF"),        # 8KB: Q^T fp16 (re+im, 2 chunks)
                ("p", 2, "SBUF"),         # 8KB x2: probs fp16 per chunk
                ("oh", 4, "SBUF"),        # 16KB: PV out fp16 (re+im, 2 chunks)
                ("z", 1, "SBUF"),         # 32KB: f32 z accumulators (2 tags)
                ("ps", 8, "PSUM"),
                ("dram", 1, "DRAM"),
            ]:
                pools[name] = ctx.enter_context(
                    tc.tile_pool(name=name, bufs=bufs, space=space))

            persist = pools["persist"]
            ident = persist.tile([P, P], F32, tag="ident")
            make_identity(nc, ident[:])

            # ---- small constants ----
            ones_f = persist.tile([1, P], F32, tag="onesf")
            nc.vector.memset(ones_f[:], 1.0)
            ones16 = persist.tile([1, P], F16, tag="ones16")
            nc.vector.tensor_copy(ones16[:], ones_f[:])
            quart_f = persist.tile([1, P], F32, tag="quartf")
            nc.vector.memset(quart_f[:], 0.25)   # bo split over 4 cores
            quart16 = persist.tile([1, P], F16, tag="quart16")
            nc.vector.tensor_copy(quart16[:], quart_f[:])
            bo_f = pools["brow"].tile([1, D], F32, tag="brow", name="bo_f")
            nc.sync.dma_start(bo_f[:], BO[:][None, :])
            bo16 = persist.tile([1, D], F16, tag="bo16")
            nc.vector.tensor_copy(bo16[:], bo_f[:])

            # ---- DRAM staging for the collective ----
            # 3 sub-ReduceScatters: quarters {0,1} merged (fires mid-h1,
            # fully overlapped), then {2} and {3} so the tail collective
            # is small. zb01 rank-r slice = rows [r*256, (r+1)*256) =
            # [q0re|q0im|q1re|q1im] x 64; zb2/zb3 rank-r slice =
            # [re|im] x 64.
            dram = pools["dram"]
            zb01 = dram.tile([2 * CH, D], F32, name="zb01")
            zb23 = dram.tile([2 * CH, D], F32, name="zb23")
            rs01_out = dram.tile([2 * P, D], F32, name="rs01out")
            rs23_out = dram.tile([2 * P, D], F32, name="rs23out")

            # ---- persistent fp16 encodes:  [128, DS, T] (D' x t layout) ----
            enc = pools["enc"]
            q_cos = enc.tile([P, DS, T], F16, tag="qc", name="q_cos")
            q_sin = enc.tile([P, DS, T], F16, tag="qs", name="q_sin")
            kv_cos = enc.tile([P, DS, T], F16, tag="kvc", name="kv_cos")
            kv_sin = enc.tile([P, DS, T], F16, tag="kvs", name="kv_sin")

            # ================= Phase 1: phasor encodes =================
            # (the only Sin-set ACT work in the kernel; q encodes run after
            # head 0's KV pass so the PE reaches the projections sooner)
            def encode(src_dram, cos_t, sin_t):
                for ch in range(NCH):
                    chsl = slice(ch * CH, (ch + 1) * CH)
                    raw_tiles = []
                    for ts in range(CH // P):
                        rt = pools["raw"].tile([P, D], F32, tag="raw")
                        nc.sync.dma_start(
                            rt[:],
                            src_dram[ch * CH + ts * P: ch * CH + (ts + 1) * P, :])
                        raw_tiles.append(rt)
                    for ds in range(DS):
                        pt = pools["ps"].tile([P, CH], F32, tag="ps")
                        for ts in range(CH // P):
                            nc.tensor.transpose(
                                pt[:, ts * P:(ts + 1) * P],
                                raw_tiles[ts][:, ds * P:(ds + 1) * P], ident[:])
                        nc.scalar.activation(sin_t[:, ds, chsl], pt[:], AF.Sin,
                                             bias=0.0, scale=PI)
                        ab = pools["nt"].tile([P, CH], F32, tag="nt")
                        nc.scalar.activation(ab[:], pt[:], AF.Abs,
                                             bias=0.0, scale=1.0)
                        nc.scalar.activation(cos_t[:, ds, chsl], ab[:], AF.Sin,
                                             bias=PI / 2, scale=-PI)

            encode(KEYVALUE, kv_cos, kv_sin)
            encode(QUERY, q_cos, q_sin)

            # ---- f32 z accumulators (summed across the 2 heads) ----
            z_re = pools["z"].tile([P, T // P, D], F32, tag="zre", name="z_re")
            z_im = pools["z"].tile([P, T // P, D], F32, tag="zim", name="z_im")

            # ================= Phase 2: per-head attention =================
            for h in range(HPC):
                # ---- biases ----
                # bq/bk as [128, DS] per-partition columns (for ACT folding)
                bq_col = pools["brow"].tile([P, DS], F32, tag="brow",
                                            name=f"bqc{h}")
                nc.sync.dma_start(bq_col[:],
                                  BQ[h].rearrange("(o p) -> p o", p=P))
                bk_col = pools["brow"].tile([P, DS], F32, tag="brow",
                                            name=f"bkc{h}")
                nc.sync.dma_start(bk_col[:],
                                  BK[h].rearrange("(o p) -> p o", p=P))
                bv_f = pools["brow"].tile([1, D], F32, tag="brow",
                                          name=f"bvr{h}")
                nc.sync.dma_start(bv_f[:], BV[h][None, :])
                bv16 = pools["brow"].tile([1, D], F16, tag="brow",
                                          name=f"bvr16{h}")
                nc.vector.tensor_copy(bv16[:], bv_f[:])

                # ---- weights -> fp16 ----
                wq16 = pools["w16"].tile([P, DS, D], F16, tag="w16",
                                         name=f"wq16_{h}")
                wk16 = pools["w16"].tile([P, DS, D], F16, tag="w16",
                                         name=f"wk16_{h}")
                wv16 = pools["w16"].tile([P, DS, D], F16, tag="w16",
                                         name=f"wv16_{h}")
                wo16 = pools["w16"].tile([P, DS, D], F16, tag="w16",
                                         name=f"wo16_{h}")
                for W_ap, w16 in ((WQ[h], wq16), (WK[h], wk16), (WV[h], wv16),
                                  (WO[h * D:(h + 1) * D, :], wo16)):
                    wf = pools["w"].tile([P, DS, D], F32, tag="wf")
                    nc.sync.dma_start(
                        wf[:], W_ap.rearrange("(o p) D -> p o D", p=P))
                    nc.vector.tensor_copy(w16[:], wf[:])

                # ---- persistent per-head tensors ----
                kt_re = pools["kt"].tile([P, DS, T], F16, tag="kt")
                kt_im = pools["kt"].tile([P, DS, T], F16, tag="kt")
                v_re = pools["v"].tile([P, T // P, D], F16, tag="v")
                v_im = pools["v"].tile([P, T // P, D], F16, tag="v")

                # ======== KV pass: K^T and V ========
                for ch in range(NCH):
                    chsl = slice(ch * CH, (ch + 1) * CH)

                    # K projection: K^T [D', t], bias folded via ACT
                    for dso in range(DS):
                        pre = pools["ps"].tile([P, CH], F32, tag="ps")
                        pim = pools["ps"].tile([P, CH], F32, tag="ps")
                        for do in range(DS):
                            nc.tensor.matmul(
                                pre[:], lhsT=wk16[:, do, dso * P:(dso + 1) * P],
                                rhs=kv_cos[:, do, chsl], start=(do == 0),
                                stop=(do == DS - 1))
                            nc.tensor.matmul(
                                pim[:], lhsT=wk16[:, do, dso * P:(dso + 1) * P],
                                rhs=kv_sin[:, do, chsl], start=(do == 0),
                                stop=(do == DS - 1))
                        b_ap = bk_col[:, dso:dso + 1]
                        nt = pools["nt"]
                        t1 = nt.tile([P, CH], F32, tag="nt")
                        nc.scalar.activation(t1[:], pre[:], AF.Square,
                                             bias=b_ap, scale=1.0)
                        t2 = nt.tile([P, CH], F32, tag="nt")
                        nc.scalar.activation(t2[:], pim[:], AF.Square,
                                             bias=0.0, scale=1.0)
                        m = nt.tile([P, CH], F32, tag="nt")
                        nc.vector.tensor_tensor(m[:], t1[:], t2[:], ALU.add)
                        sq = nt.tile([P, CH], F32, tag="nt")
                        nc.scalar.activation(sq[:], m[:], AF.Sqrt,
                                             bias=0.0, scale=1.0)
                        n = nt.tile([P, CH], F32, tag="nt")
                        nc.vector.reciprocal_approx_fast(n[:], sq[:])
                        nc.vector.scalar_tensor_tensor(
                            kt_re[:, dso, chsl], pre[:], b_ap, n[:],
                            ALU.add, ALU.mult)
                        nc.vector.tensor_tensor(kt_im[:, dso, chsl], pim[:],
                                                n[:], ALU.mult)

                    # V projection: V [t, D], bias as K=1 matmul
                    for tb in range(CH // P):
                        tsl = slice(ch * CH + tb * P, ch * CH + (tb + 1) * P)
                        pre = pools["ps"].tile([P, D], F32, tag="ps")
                        pim = pools["ps"].tile([P, D], F32, tag="ps")
                        for do in range(DS):
                            nc.tensor.matmul(
                                pre[:], lhsT=kv_cos[:, do, tsl],
                                rhs=wv16[:, do, :], start=(do == 0), stop=False)
                            nc.tensor.matmul(
                                pim[:], lhsT=kv_sin[:, do, tsl],
                                rhs=wv16[:, do, :], start=(do == 0),
                                stop=(do == DS - 1))
                        nc.tensor.matmul(
                            pre[:], lhsT=ones16[:], rhs=bv16[:],
                            start=False, stop=True)
                        trow = ch * (CH // P) + tb
                        nt = pools["nt"]
                        t1 = nt.tile([P, D], F32, tag="nt")
                        nc.scalar.activation(t1[:], pre[:], AF.Square,
                                             bias=0.0, scale=1.0)
                        t2 = nt.tile([P, D], F32, tag="nt")
                        nc.scalar.activation(t2[:], pim[:], AF.Square,
                                             bias=0.0, scale=1.0)
                        m = nt.tile([P, D], F32, tag="nt")
                        nc.vector.tensor_tensor(m[:], t1[:], t2[:], ALU.add)
                        sq = nt.tile([P, D], F32, tag="nt")
                        nc.scalar.activation(sq[:], m[:], AF.Sqrt,
                                             bias=0.0, scale=1.0)
                        n = nt.tile([P, D], F32, tag="nt")
                        nc.vector.reciprocal_approx_fast(n[:], sq[:])
                        nc.vector.tensor_tensor(v_re[:, trow, :], pre[:], n[:],
                                                ALU.mult)
                        nc.vector.tensor_tensor(v_im[:, trow, :], pim[:], n[:],
                                                ALU.mult)

                # ======== Q projections (both chunks up front) ========
                qts = {}
                for ch in range(NCH):
                    qsl = slice(ch * CH, (ch + 1) * CH)
                    qt_re = pools["qt"].tile([P, DS, CH], F16, tag="qt")
                    qt_im = pools["qt"].tile([P, DS, CH], F16, tag="qt")
                    qts[ch] = (qt_re, qt_im)
                    for dso in range(DS):
                        pre = pools["ps"].tile([P, CH], F32, tag="ps")
                        pim = pools["ps"].tile([P, CH], F32, tag="ps")
                        for do in range(DS):
                            nc.tensor.matmul(
                                pre[:], lhsT=wq16[:, do, dso * P:(dso + 1) * P],
                                rhs=q_cos[:, do, qsl], start=(do == 0),
                                stop=(do == DS - 1))
                            nc.tensor.matmul(
                                pim[:], lhsT=wq16[:, do, dso * P:(dso + 1) * P],
                                rhs=q_sin[:, do, qsl], start=(do == 0),
                                stop=(do == DS - 1))
                        b_ap = bq_col[:, dso:dso + 1]
                        nt = pools["nt"]
                        t1 = nt.tile([P, CH], F32, tag="nt")
                        nc.scalar.activation(t1[:], pre[:], AF.Square,
                                             bias=b_ap, scale=1.0)
                        t2 = nt.tile([P, CH], F32, tag="nt")
                        nc.scalar.activation(t2[:], pim[:], AF.Square,
                                             bias=0.0, scale=1.0)
                        m = nt.tile([P, CH], F32, tag="nt")
                        nc.vector.tensor_tensor(m[:], t1[:], t2[:], ALU.add)
                        sq = nt.tile([P, CH], F32, tag="nt")
                        nc.scalar.activation(sq[:], m[:], AF.Sqrt,
                                             bias=0.0, scale=1.0)
                        n = nt.tile([P, CH], F32, tag="nt")
                        nc.vector.reciprocal_approx_fast(n[:], sq[:])
                        nc.vector.scalar_tensor_tensor(
                            qt_re[:, dso, :], pre[:], b_ap, n[:],
                            ALU.add, ALU.mult)
                        nc.vector.tensor_tensor(qt_im[:, dso, :], pim[:],
                                                n[:], ALU.mult)

                # ======== attention + final dense per 512-chunk ========
                for ch in range(NCH):
                    qsl = slice(ch * CH, (ch + 1) * CH)
                    qt_re, qt_im = qts[ch]

                    # scores + exp -> P^T [Tkv, tq-chunk] fp16
                    pt_all = pools["p"].tile([P, T // P, CH], F16, tag="p")
                    for to in range(T // P):
                        ps_s = pools["ps"].tile([P, CH], F32, tag="ps")
                        for do in range(DS):
                            nc.tensor.matmul(
                                ps_s[:], lhsT=kt_re[:, do, to * P:(to + 1) * P],
                                rhs=qt_re[:, do, :], start=(do == 0), stop=False)
                        for do in range(DS):
                            nc.tensor.matmul(
                                ps_s[:], lhsT=kt_im[:, do, to * P:(to + 1) * P],
                                rhs=qt_im[:, do, :], start=False,
                                stop=(do == DS - 1))
                        nc.scalar.activation(pt_all[:, to, :], ps_s[:], AF.Exp,
                                             bias=0.0, scale=1.0 / D)

                    # PV: O^T [D', tq-chunk], two groups of 2 D'-slices
                    oh_re = pools["oh"].tile([P, DS, CH], F16, tag="oh")
                    oh_im = pools["oh"].tile([P, DS, CH], F16, tag="oh")
                    for grp in range(2):
                        ps_tiles = {}
                        for dso in (2 * grp, 2 * grp + 1):
                            ps_tiles[(dso, 0)] = pools["ps"].tile(
                                [P, CH], F32, tag="ps", name=f"pv_{h}_{ch}_{dso}_re")
                            ps_tiles[(dso, 1)] = pools["ps"].tile(
                                [P, CH], F32, tag="ps", name=f"pv_{h}_{ch}_{dso}_im")
                        for to in range(T // P):
                            for dso in (2 * grp, 2 * grp + 1):
                                nc.tensor.matmul(
                                    ps_tiles[(dso, 0)][:],
                                    lhsT=v_re[:, to, dso * P:(dso + 1) * P],
                                    rhs=pt_all[:, to, :], start=(to == 0),
                                    stop=(to == T // P - 1))
                                nc.tensor.matmul(
                                    ps_tiles[(dso, 1)][:],
                                    lhsT=v_im[:, to, dso * P:(dso + 1) * P],
                                    rhs=pt_all[:, to, :], start=(to == 0),
                                    stop=(to == T // P - 1))
                        for dso in (2 * grp, 2 * grp + 1):
                            pre, pim = ps_tiles[(dso, 0)], ps_tiles[(dso, 1)]
                            nt = pools["nt"]
                            t1 = nt.tile([P, CH], F32, tag="nt")
                            nc.scalar.activation(t1[:], pre[:], AF.Square,
                                                 bias=0.0, scale=1.0)
                            t2 = nt.tile([P, CH], F32, tag="nt")
                            nc.scalar.activation(t2[:], pim[:], AF.Square,
                                                 bias=0.0, scale=1.0)
                            m = nt.tile([P, CH], F32, tag="nt")
                            nc.vector.tensor_tensor(m[:], t1[:], t2[:], ALU.add)
                            sq = nt.tile([P, CH], F32, tag="nt")
                            nc.scalar.activation(sq[:], m[:], AF.Sqrt,
                                                 bias=0.0, scale=1.0)
                            n = nt.tile([P, CH], F32, tag="nt")
                            nc.vector.reciprocal_approx_fast(n[:], sq[:])
                            nc.vector.tensor_tensor(oh_re[:, dso, :], pre[:],
                                                    n[:], ALU.mult)
                            nc.vector.tensor_tensor(oh_im[:, dso, :], pim[:],
                                                    n[:], ALU.mult)

                    # final dense partial: Z [tq, D] += Ohat^T.T @ wo_h
                    for ts in range(CH // P):
                        pzre = pools["ps"].tile([P, D], F32, tag="ps")
                        pzim = pools["ps"].tile([P, D], F32, tag="ps")
                        for do in range(DS):
                            nc.tensor.matmul(
                                pzre[:], lhsT=oh_re[:, do, ts * P:(ts + 1) * P],
                                rhs=wo16[:, do, :], start=(do == 0),
                                stop=(do == DS - 1 and h != 0))
                            nc.tensor.matmul(
                                pzim[:], lhsT=oh_im[:, do, ts * P:(ts + 1) * P],
                                rhs=wo16[:, do, :], start=(do == 0),
                                stop=(do == DS - 1))
                        if h == 0:
                            nc.tensor.matmul(
                                pzre[:], lhsT=quart16[:], rhs=bo16[:],
                                start=False, stop=True)
                        trow = ch * (CH // P) + ts
                        if h == 0:
                            nc.scalar.copy(z_re[:, trow, :], pzre[:])
                            nc.scalar.copy(z_im[:, trow, :], pzim[:])
                        else:
                            nc.vector.tensor_tensor(z_re[:, trow, :],
                                                    z_re[:, trow, :], pzre[:],
                                                    ALU.add)
                            nc.vector.tensor_tensor(z_im[:, trow, :],
                                                    z_im[:, trow, :], pzim[:],
                                                    ALU.add)
                            tq0 = ch * CH + ts * P
                            qq = tq0 // 256
                            r0 = 2 * ((tq0 % 256) // P)  # 0 or 2
                            dst = zb01 if qq <= 1 else zb23
                            qloc = qq % 2
                            for half in range(2):
                                r_ = r0 + half
                                src = slice(half * 64, (half + 1) * 64)
                                row = r_ * 256 + qloc * 128
                                nc.sync.dma_start(
                                    dst[row: row + 64, :],
                                    z_re[src, trow, :])
                                nc.sync.dma_start(
                                    dst[row + 64: row + 128, :],
                                    z_im[src, trow, :])
                            # fire per chunk: {0,1} after h1 chunk 0 (overlaps
                            # chunk 1), {2,3} at the end
                            if ts == CH // P - 1:
                                ins_, outs_ = ((zb01, rs01_out) if ch == 0
                                               else (zb23, rs23_out))
                                nc.gpsimd.collective_compute(
                                    "ReduceScatter", ALU.add,
                                    replica_groups=[[0, 1, 2, 3], [4, 5, 6, 7]],
                                    ins=[ins_.opt()],
                                    outs=[outs_.opt()],
                                )

            # ======== atan2(zim, zre)/pi, quarters batched in pairs ========
            for pp in range(2):
                zre_t = pools["nt"].tile([P, D], F32, tag="nt", name=f"zre{pp}")
                zim_t = pools["nt"].tile([P, D], F32, tag="nt", name=f"zim{pp}")
                rs_out = rs01_out if pp == 0 else rs23_out
                nc.sync.dma_start(zre_t[0:64, :], rs_out[0:64, :])
                nc.sync.dma_start(zim_t[0:64, :], rs_out[64:128, :])
                nc.sync.dma_start(zre_t[64:128, :], rs_out[128:192, :])
                nc.sync.dma_start(zim_t[64:128, :], rs_out[192:256, :])
                nt = pools["nt"]

                def ft(nm, pp=pp):
                    return nt.tile([P, D], F32, tag="nt", name=f"{nm}{pp}")
                t1, t2, m, az = ft("f1"), ft("f2"), ft("f3"), ft("f5")
                den1, r1, ta0, ta = ft("f6"), ft("f7"), ft("f8"), ft("f9")
                num2, r2, tb0, tb = ft("fa"), ft("fb"), ft("fc"), ft("fd")
                ata, atb, mask = ft("fe"), ft("ff"), ft("fg")
                dsel, md, sel, outt = ft("fh"), ft("fi"), ft("fj"), ft("fk")
                for hw in range(2):
                    c = slice(hw * (D // 2), (hw + 1) * (D // 2))
                    zre, zim = zre_t[:, c], zim_t[:, c]
                    nc.scalar.activation(t1[:, c], zre, AF.Square,
                                         bias=0.0, scale=1.0)
                    nc.vector.tensor_tensor(t2[:, c], zim, zim, ALU.mult)
                    nc.vector.tensor_tensor(m[:, c], t1[:, c], t2[:, c],
                                            ALU.add)
                    nc.scalar.activation(az[:, c], m[:, c], AF.Sqrt,
                                         bias=0.0, scale=1.0)
                    nc.vector.tensor_tensor(den1[:, c], az[:, c], zre, ALU.add)
                    nc.vector.reciprocal_approx_fast(r1[:, c], den1[:, c])
                    nc.vector.tensor_tensor(ta0[:, c], zim, r1[:, c], ALU.mult)
                    nc.vector.tensor_scalar(ta[:, c], ta0[:, c], 1e8, -1e8,
                                            ALU.min, ALU.max)
                    nc.vector.tensor_tensor(num2[:, c], az[:, c], zre,
                                            ALU.subtract)
                    nc.vector.reciprocal_approx_fast(r2[:, c], zim)
                    nc.vector.tensor_tensor(tb0[:, c], num2[:, c], r2[:, c],
                                            ALU.mult)
                    nc.vector.tensor_scalar(tb[:, c], tb0[:, c], 1e8, -1e8,
                                            ALU.min, ALU.max)
                    nc.scalar.activation(ata[:, c], ta[:, c], AF.Arctan,
                                         bias=0.0, scale=1.0)
                    nc.scalar.activation(atb[:, c], tb[:, c], AF.Arctan,
                                         bias=0.0, scale=1.0)
                    nc.vector.tensor_scalar(mask[:, c], zre, 0.0, None,
                                            ALU.is_ge)
                    nc.vector.tensor_tensor(dsel[:, c], ata[:, c], atb[:, c],
                                            ALU.subtract)
                    nc.vector.tensor_tensor(md[:, c], mask[:, c], dsel[:, c],
                                            ALU.mult)
                    nc.vector.tensor_tensor(sel[:, c], atb[:, c], md[:, c],
                                            ALU.add)
                    nc.vector.tensor_scalar(outt[:, c], sel[:, c], 2.0 / PI,
                                            None, ALU.mult)
                nc.sync.dma_start(OUT[pp * P:(pp + 1) * P, :], outt[:, :])

    nc.finalize()
    return nc


_NC_CACHE = {}


def _get_nc():
    if "nc" not in _NC_CACHE:
        _NC_CACHE["nc"] = build()
    return _NC_CACHE["nc"]


def kernel(**inputs):
    query = np.ascontiguousarray(np.asarray(inputs["query"], dtype=np.float32))
    keyvalue = np.ascontiguousarray(np.asarray(inputs["keyvalue"], dtype=np.float32))
    wq = np.asarray(inputs["wq"], dtype=np.float32)
    wk = np.asarray(inputs["wk"], dtype=np.float32)
    wv = np.asarray(inputs["wv"], dtype=np.float32)
    bq = np.asarray(inputs["bq"], dtype=np.float32)
    bk = np.asarray(inputs["bk"], dtype=np.float32)
    bv = np.asarray(inputs["bv"], dtype=np.float32)
    wo = np.asarray(inputs["wo"], dtype=np.float32)
    bo = np.asarray(inputs["bo"], dtype=np.float32)

    in_maps = []
    for c in range(N_CORES):
        b, g = c // 4, c % 4
        h0 = g * HPC
        in_maps.append({
            "query": query[b],
            "keyvalue": keyvalue[b],
            "wq": np.ascontiguousarray(wq[h0:h0 + HPC]),
            "wk": np.ascontiguousarray(wk[h0:h0 + HPC]),
            "wv": np.ascontiguousarray(wv[h0:h0 + HPC]),
            "bq": np.ascontiguousarray(bq[h0:h0 + HPC]),
            "bk": np.ascontiguousarray(bk[h0:h0 + HPC]),
            "bv": np.ascontiguousarray(bv[h0:h0 + HPC]),
            "wo": np.ascontiguousarray(wo[h0 * D:(h0 + HPC) * D]),
            "bo": bo,
        })

    nc = _get_nc()
    res = run_bass_kernel_spmd(nc, in_maps, core_ids=list(range(N_CORES)))
    _NC_CACHE["last_results"] = res
    out = np.empty((B, T, D), np.float32)
    for c in range(N_CORES):
        b, g = c // 4, c % 4
        o = res.results[c]["out"]          # [256, 512]: 4 quarters x 64 rows
        for qq in range(4):
            out[b, qq * 256 + g * 64: qq * 256 + (g + 1) * 64, :] = \
                o[qq * 64:(qq + 1) * 64, :]
    return out


# revision 29
# speedup vs baseline: 1.1927x; 1.1927x over previous
"""Distributed Trainium2 Bass kernel for the phasor attention problem
(nn_Attention_17798344475248).

Sharding: 8 cores = 2 batches x 4 head-groups (2 heads each). Each core
computes its batch's Q/K/V projections for its 2 heads, phasor attention,
and a partial final-dense output; partials are summed with 4 pipelined
4-rank ReduceScatters per batch group; each core finishes atan2 on its
4x64-row slices of the output.

Math notes (vs reference.py):
- phasor_encode(phasor_act(z)) == z/|z|  -> normalize instead of atan2+cos/sin
- softmax max-subtract and sum-normalization cancel in the final angle
  (positive per-row scale), so softmax reduces to exp(s/d)
- complex bias (real) folds into the ACT Square/rescale passes for Q/K
  (per-partition bias) and stays a K=1 outer-product matmul for V / final
- all matmul operands are fp16 (10-bit mantissa, 1 PE cycle/row, fast
  weight load) with f32 PSUM accumulation
- phasor encodes (the only Sin-set ACT work) run in one pipelined phase
  at the start and stay resident in SBUF as fp16
"""
import sys

sys.path.insert(0, "/opt/trn_rl_repo")

import numpy as np

import concourse.bass as bass
import concourse.tile as tile
from concourse import bacc, mybir
from concourse.bass_utils import run_bass_kernel_spmd
from concourse.masks import make_identity

F32 = mybir.dt.float32
F16 = mybir.dt.float16
AF = mybir.ActivationFunctionType
ALU = mybir.AluOpType
PI = float(np.pi)

B, T, D, H = 2, 1024, 512, 8
P = 128
DS = D // P          # 4 partition-slices of the model dim
CH = 512             # chunk width along t (both q and kv passes)
NCH = T // CH        # 2 chunks
N_CORES = 8
HPC = 2              # heads per core


def build(debug=False):
    nc = bacc.Bacc("TRN2", target_bir_lowering=False, debug=False,
                   num_devices=N_CORES)
    cpi2 = nc.alloc_sbuf_tensor("const-f32-pi2", [P, 1], F32)
    nc.gpsimd.memset(cpi2.ap(), PI / 2)
    nc.const_aps.aps[(F32, PI / 2)] = cpi2.ap()
    nc.all_engine_barrier()

    # ---- I/O ----
    QUERY = nc.dram_tensor("query", [T, D], F32, kind="ExternalInput")
    KEYVALUE = nc.dram_tensor("keyvalue", [T, D], F32, kind="ExternalInput")
    WQ = nc.dram_tensor("wq", [HPC, D, D], F32, kind="ExternalInput")
    WK = nc.dram_tensor("wk", [HPC, D, D], F32, kind="ExternalInput")
    WV = nc.dram_tensor("wv", [HPC, D, D], F32, kind="ExternalInput")
    BQ = nc.dram_tensor("bq", [HPC, D], F32, kind="ExternalInput")
    BK = nc.dram_tensor("bk", [HPC, D], F32, kind="ExternalInput")
    BV = nc.dram_tensor("bv", [HPC, D], F32, kind="ExternalInput")
    WO = nc.dram_tensor("wo", [HPC * D, D], F32, kind="ExternalInput")
    BO = nc.dram_tensor("bo", [D], F32, kind="ExternalInput")
    OUT = nc.dram_tensor("out", [T // 4, D], F32, kind="ExternalOutput")

    with tile.TileContext(nc) as tc:
        import contextlib
        with contextlib.ExitStack() as ctx:
            pools = {}
            for name, bufs, space in [
                ("persist", 1, "SBUF"),
                ("raw", 4, "SBUF"),       # 2KB x4 raw input tiles
                ("nt", 8, "SBUF"),        # 2KB x8 norm/atan2 temps
                ("w", 1, "SBUF"),         # 8KB f32 weight staging
                ("w16", 4, "SBUF"),       # 2KB x4 fp16 weights (per head)
                ("brow", 6, "SBUF"),      # small bias rows/cols
                ("enc", 1, "SBUF"),       # 32KB: q/kv cos/sin fp16 (4 tags)
                ("kt", 2, "SBUF"),        # 16KB: K^T fp16 (re+im)
                ("v", 2, "SBUF"),         # 16KB: V fp16 (re+im)
                ("qt", 4, "SBUF"),        # 8KB: Q^T fp16 (re+im, 2 chunks)
                ("p", 2, "SBUF"),         # 8KB x2: probs fp16 per chunk
                ("oh", 4, "SBUF"),        # 16KB: PV out fp16 (re+im, 2 chunks)
                ("z", 1, "SBUF"),         # 32KB: f32 z accumulators (2 tags)
                ("ps", 8, "PSUM"),
                ("dram", 1, "DRAM"),
            ]:
                pools[name] = ctx.enter_context(
                    tc.tile_pool(name=name, bufs=bufs, space=space))

            persist = pools["persist"]
            ident = persist.tile([P, P], F32, tag="ident")
            make_identity(nc, ident[:])

            # ---- small constants ----
            ones_f = persist.tile([1, P], F32, tag="onesf")
            nc.vector.memset(ones_f[:], 1.0)
            ones16 = persist.tile([1, P], F16, tag="ones16")
            nc.vector.tensor_copy(ones16[:], ones_f[:])
            quart_f = persist.tile([1, P], F32, tag="quartf")
            nc.vector.memset(quart_f[:], 0.25)   # bo split over 4 cores
            quart16 = persist.tile([1, P], F16, tag="quart16")
            nc.vector.tensor_copy(quart16[:], quart_f[:])
            bo_f = pools["brow"].tile([1, D], F32, tag="brow", name="bo_f")
            nc.sync.dma_start(bo_f[:], BO[:][None, :])
            bo16 = persist.tile([1, D], F16, tag="bo16")
            nc.vector.tensor_copy(bo16[:], bo_f[:])

            # ---- DRAM staging for the collective ----
            # 3 sub-ReduceScatters: quarters {0,1} merged (fires mid-h1,
            # fully overlapped), then {2} and {3} so the tail collective
            # is small. zb01 rank-r slice = rows [r*256, (r+1)*256) =
            # [q0re|q0im|q1re|q1im] x 64; zb2/zb3 rank-r slice =
            # [re|im] x 64.
            dram = pools["dram"]
            zbs = [dram.tile([CH, D], F32, name=f"zb{q}") for q in range(4)]
            rs_outs = [dram.tile([P, D], F32, name=f"rsout{q}")
                       for q in range(4)]

            # ---- persistent fp16 encodes:  [128, DS, T] (D' x t layout) ----
            enc = pools["enc"]
            q_cos = enc.tile([P, DS, T], F16, tag="qc", name="q_cos")
            q_sin = enc.tile([P, DS, T], F16, tag="qs", name="q_sin")
            kv_cos = enc.tile([P, DS, T], F16, tag="kvc", name="kv_cos")
            kv_sin = enc.tile([P, DS, T], F16, tag="kvs", name="kv_sin")

            # ================= Phase 1: phasor encodes =================
            # (the only Sin-set ACT work in the kernel; q encodes run after
            # head 0's KV pass so the PE reaches the projections sooner)
            def encode(src_dram, cos_t, sin_t):
                for ch in range(NCH):
                    chsl = slice(ch * CH, (ch + 1) * CH)
                    raw_tiles = []
                    for ts in range(CH // P):
                        rt = pools["raw"].tile([P, D], F32, tag="raw")
                        nc.sync.dma_start(
                            rt[:],
                            src_dram[ch * CH + ts * P: ch * CH + (ts + 1) * P, :])
                        raw_tiles.append(rt)
                    for ds in range(DS):
                        pt = pools["ps"].tile([P, CH], F32, tag="ps")
                        for ts in range(CH // P):
                            nc.tensor.transpose(
                                pt[:, ts * P:(ts + 1) * P],
                                raw_tiles[ts][:, ds * P:(ds + 1) * P], ident[:])
                        nc.scalar.activation(sin_t[:, ds, chsl], pt[:], AF.Sin,
                                             bias=0.0, scale=PI)
                        ab = pools["nt"].tile([P, CH], F32, tag="nt")
                        nc.scalar.activation(ab[:], pt[:], AF.Abs,
                                             bias=0.0, scale=1.0)
                        nc.scalar.activation(cos_t[:, ds, chsl], ab[:], AF.Sin,
                                             bias=PI / 2, scale=-PI)

            encode(KEYVALUE, kv_cos, kv_sin)
            encode(QUERY, q_cos, q_sin)

            # ---- f32 z accumulators (summed across the 2 heads) ----
            z_re = pools["z"].tile([P, T // P, D], F32, tag="zre", name="z_re")
            z_im = pools["z"].tile([P, T // P, D], F32, tag="zim", name="z_im")

            # ================= Phase 2: per-head attention =================
            for h in range(HPC):
                # ---- biases ----
                # bq/bk as [128, DS] per-partition columns (for ACT folding)
                bq_col = pools["brow"].tile([P, DS], F32, tag="brow",
                                            name=f"bqc{h}")
                nc.sync.dma_start(bq_col[:],
                                  BQ[h].rearrange("(o p) -> p o", p=P))
                bk_col = pools["brow"].tile([P, DS], F32, tag="brow",
                                            name=f"bkc{h}")
                nc.sync.dma_start(bk_col[:],
                                  BK[h].rearrange("(o p) -> p o", p=P))
                bv_f = pools["brow"].tile([1, D], F32, tag="brow",
                                          name=f"bvr{h}")
                nc.sync.dma_start(bv_f[:], BV[h][None, :])
                bv16 = pools["brow"].tile([1, D], F16, tag="brow",
                                          name=f"bvr16{h}")
                nc.vector.tensor_copy(bv16[:], bv_f[:])

                # ---- weights -> fp16 ----
                wq16 = pools["w16"].tile([P, DS, D], F16, tag="w16",
                                         name=f"wq16_{h}")
                wk16 = pools["w16"].tile([P, DS, D], F16, tag="w16",
                                         name=f"wk16_{h}")
                wv16 = pools["w16"].tile([P, DS, D], F16, tag="w16",
                                         name=f"wv16_{h}")
                wo16 = pools["w16"].tile([P, DS, D], F16, tag="w16",
                                         name=f"wo16_{h}")
                for W_ap, w16 in ((WQ[h], wq16), (WK[h], wk16), (WV[h], wv16),
                                  (WO[h * D:(h + 1) * D, :], wo16)):
                    wf = pools["w"].tile([P, DS, D], F32, tag="wf")
                    nc.sync.dma_start(
                        wf[:], W_ap.rearrange("(o p) D -> p o D", p=P))
                    nc.vector.tensor_copy(w16[:], wf[:])

                # ---- persistent per-head tensors ----
                kt_re = pools["kt"].tile([P, DS, T], F16, tag="kt")
                kt_im = pools["kt"].tile([P, DS, T], F16, tag="kt")
                v_re = pools["v"].tile([P, T // P, D], F16, tag="v")
                v_im = pools["v"].tile([P, T // P, D], F16, tag="v")

                # ======== KV pass: K^T and V ========
                for ch in range(NCH):
                    chsl = slice(ch * CH, (ch + 1) * CH)

                    # K projection: K^T [D', t], bias folded via ACT
                    for dso in range(DS):
                        pre = pools["ps"].tile([P, CH], F32, tag="ps")
                        pim = pools["ps"].tile([P, CH], F32, tag="ps")
                        for do in range(DS):
                            nc.tensor.matmul(
                                pre[:], lhsT=wk16[:, do, dso * P:(dso + 1) * P],
                                rhs=kv_cos[:, do, chsl], start=(do == 0),
                                stop=(do == DS - 1))
                            nc.tensor.matmul(
                                pim[:], lhsT=wk16[:, do, dso * P:(dso + 1) * P],
                                rhs=kv_sin[:, do, chsl], start=(do == 0),
                                stop=(do == DS - 1))
                        b_ap = bk_col[:, dso:dso + 1]
                        nt = pools["nt"]
                        t1 = nt.tile([P, CH], F32, tag="nt")
                        nc.scalar.activation(t1[:], pre[:], AF.Square,
                                             bias=b_ap, scale=1.0)
                        t2 = nt.tile([P, CH], F32, tag="nt")
                        nc.scalar.activation(t2[:], pim[:], AF.Square,
                                             bias=0.0, scale=1.0)
                        nc.vector.tensor_tensor(t1[:], t1[:], t2[:], ALU.add)
                        nc.scalar.activation(t2[:], t1[:], AF.Sqrt,
                                             bias=0.0, scale=1.0)
                        n = t1
                        nc.vector.reciprocal_approx_fast(n[:], t2[:])
                        nc.vector.scalar_tensor_tensor(
                            kt_re[:, dso, chsl], pre[:], b_ap, n[:],
                            ALU.add, ALU.mult)
                        nc.vector.tensor_tensor(kt_im[:, dso, chsl], pim[:],
                                                n[:], ALU.mult)

                    # V projection: V [t, D], bias as K=1 matmul
                    for tb in range(CH // P):
                        tsl = slice(ch * CH + tb * P, ch * CH + (tb + 1) * P)
                        pre = pools["ps"].tile([P, D], F32, tag="ps")
                        pim = pools["ps"].tile([P, D], F32, tag="ps")
                        for do in range(DS):
                            nc.tensor.matmul(
                                pre[:], lhsT=kv_cos[:, do, tsl],
                                rhs=wv16[:, do, :], start=(do == 0), stop=False)
                            nc.tensor.matmul(
                                pim[:], lhsT=kv_sin[:, do, tsl],
                                rhs=wv16[:, do, :], start=(do == 0),
                                stop=(do == DS - 1))
                        nc.tensor.matmul(
                            pre[:], lhsT=ones16[:], rhs=bv16[:],
                            start=False, stop=True)
                        trow = ch * (CH // P) + tb
                        nt = pools["nt"]
                        t1 = nt.tile([P, D], F32, tag="nt")
                        nc.scalar.activation(t1[:], pre[:], AF.Square,
                                             bias=0.0, scale=1.0)
                        t2 = nt.tile([P, D], F32, tag="nt")
                        nc.scalar.activation(t2[:], pim[:], AF.Square,
                                             bias=0.0, scale=1.0)
                        nc.vector.tensor_tensor(t1[:], t1[:], t2[:], ALU.add)
                        nc.scalar.activation(t2[:], t1[:], AF.Sqrt,
                                             bias=0.0, scale=1.0)
                        n = t1
                        nc.vector.reciprocal_approx_fast(n[:], t2[:])
                        nc.vector.tensor_tensor(v_re[:, trow, :], pre[:], n[:],
                                                ALU.mult)
                        nc.vector.tensor_tensor(v_im[:, trow, :], pim[:], n[:],
                                                ALU.mult)

                # ======== Q projections (both chunks up front) ========
                qts = {}
                for ch in range(NCH):
                    qsl = slice(ch * CH, (ch + 1) * CH)
                    qt_re = pools["qt"].tile([P, DS, CH], F16, tag="qt")
                    qt_im = pools["qt"].tile([P, DS, CH], F16, tag="qt")
                    qts[ch] = (qt_re, qt_im)
                    for dso in range(DS):
                        pre = pools["ps"].tile([P, CH], F32, tag="ps")
                        pim = pools["ps"].tile([P, CH], F32, tag="ps")
                        for do in range(DS):
                            nc.tensor.matmul(
                                pre[:], lhsT=wq16[:, do, dso * P:(dso + 1) * P],
                                rhs=q_cos[:, do, qsl], start=(do == 0),
                                stop=(do == DS - 1))
                            nc.tensor.matmul(
                                pim[:], lhsT=wq16[:, do, dso * P:(dso + 1) * P],
                                rhs=q_sin[:, do, qsl], start=(do == 0),
                                stop=(do == DS - 1))
                        b_ap = bq_col[:, dso:dso + 1]
                        nt = pools["nt"]
                        t1 = nt.tile([P, CH], F32, tag="nt")
                        nc.scalar.activation(t1[:], pre[:], AF.Square,
                                             bias=b_ap, scale=1.0)
                        t2 = nt.tile([P, CH], F32, tag="nt")
                        nc.scalar.activation(t2[:], pim[:], AF.Square,
                                             bias=0.0, scale=1.0)
                        nc.vector.tensor_tensor(t1[:], t1[:], t2[:], ALU.add)
                        nc.scalar.activation(t2[:], t1[:], AF.Sqrt,
                                             bias=0.0, scale=1.0)
                        n = t1
                        nc.vector.reciprocal_approx_fast(n[:], t2[:])
                        nc.vector.scalar_tensor_tensor(
                            qt_re[:, dso, :], pre[:], b_ap, n[:],
                            ALU.add, ALU.mult)
                        nc.vector.tensor_tensor(qt_im[:, dso, :], pim[:],
                                                n[:], ALU.mult)

                # ======== attention + final dense per 512-chunk ========
                for ch in range(NCH):
                    qsl = slice(ch * CH, (ch + 1) * CH)
                    qt_re, qt_im = qts[ch]

                    # scores + exp -> P^T [Tkv, tq-chunk] fp16
                    pt_all = pools["p"].tile([P, T // P, CH], F16, tag="p")
                    for to in range(T // P):
                        ps_s = pools["ps"].tile([P, CH], F32, tag="ps")
                        for do in range(DS):
                            nc.tensor.matmul(
                                ps_s[:], lhsT=kt_re[:, do, to * P:(to + 1) * P],
                                rhs=qt_re[:, do, :], start=(do == 0), stop=False)
                        for do in range(DS):
                            nc.tensor.matmul(
                                ps_s[:], lhsT=kt_im[:, do, to * P:(to + 1) * P],
                                rhs=qt_im[:, do, :], start=False,
                                stop=(do == DS - 1))
                        nc.scalar.activation(pt_all[:, to, :], ps_s[:], AF.Exp,
                                             bias=0.0, scale=1.0 / D)

                    # PV: O^T [D', tq-chunk], two groups of 2 D'-slices
                    oh_re = pools["oh"].tile([P, DS, CH], F16, tag="oh")
                    oh_im = pools["oh"].tile([P, DS, CH], F16, tag="oh")
                    for grp in range(2):
                        ps_tiles = {}
                        for dso in (2 * grp, 2 * grp + 1):
                            ps_tiles[(dso, 0)] = pools["ps"].tile(
                                [P, CH], F32, tag="ps", name=f"pv_{h}_{ch}_{dso}_re")
                            ps_tiles[(dso, 1)] = pools["ps"].tile(
                                [P, CH], F32, tag="ps", name=f"pv_{h}_{ch}_{dso}_im")
                        for to in range(T // P):
                            for dso in (2 * grp, 2 * grp + 1):
                                nc.tensor.matmul(
                                    ps_tiles[(dso, 0)][:],
                                    lhsT=v_re[:, to, dso * P:(dso + 1) * P],
                                    rhs=pt_all[:, to, :], start=(to == 0),
                                    stop=(to == T // P - 1))
                                nc.tensor.matmul(
                                    ps_tiles[(dso, 1)][:],
                                    lhsT=v_im[:, to, dso * P:(dso + 1) * P],
                                    rhs=pt_all[:, to, :], start=(to == 0),
                                    stop=(to == T // P - 1))
                        for dso in (2 * grp, 2 * grp + 1):
                            pre, pim = ps_tiles[(dso, 0)], ps_tiles[(dso, 1)]
                            nt = pools["nt"]
                            t1 = nt.tile([P, CH], F32, tag="nt")
                            nc.scalar.activation(t1[:], pre[:], AF.Square,
                                                 bias=0.0, scale=1.0)
                            t2 = nt.tile([P, CH], F32, tag="nt")
                            nc.scalar.activation(t2[:], pim[:], AF.Square,
                                                 bias=0.0, scale=1.0)
                            nc.vector.tensor_tensor(t1[:], t1[:], t2[:],
                                                    ALU.add)
                            nc.scalar.activation(t2[:], t1[:], AF.Sqrt,
                                                 bias=0.0, scale=1.0)
                            n = t1
                            nc.vector.reciprocal_approx_fast(n[:], t2[:])
                            nc.vector.tensor_tensor(oh_re[:, dso, :], pre[:],
                                                    n[:], ALU.mult)
                            nc.vector.tensor_tensor(oh_im[:, dso, :], pim[:],
                                                    n[:], ALU.mult)

                    # final dense partial: Z [tq, D] += Ohat^T.T @ wo_h
                    for ts in range(CH // P):
                        pzre = pools["ps"].tile([P, D], F32, tag="ps")
                        pzim = pools["ps"].tile([P, D], F32, tag="ps")
                        for do in range(DS):
                            nc.tensor.matmul(
                                pzre[:], lhsT=oh_re[:, do, ts * P:(ts + 1) * P],
                                rhs=wo16[:, do, :], start=(do == 0),
                                stop=(do == DS - 1 and h != 0))
                            nc.tensor.matmul(
                                pzim[:], lhsT=oh_im[:, do, ts * P:(ts + 1) * P],
                                rhs=wo16[:, do, :], start=(do == 0),
                                stop=(do == DS - 1))
                        if h == 0:
                            nc.tensor.matmul(
                                pzre[:], lhsT=quart16[:], rhs=bo16[:],
                                start=False, stop=True)
                        trow = ch * (CH // P) + ts
                        if h == 0:
                            nc.scalar.copy(z_re[:, trow, :], pzre[:])
                            nc.scalar.copy(z_im[:, trow, :], pzim[:])
                        else:
                            nc.vector.tensor_tensor(z_re[:, trow, :],
                                                    z_re[:, trow, :], pzre[:],
                                                    ALU.add)
                            nc.vector.tensor_tensor(z_im[:, trow, :],
                                                    z_im[:, trow, :], pzim[:],
                                                    ALU.add)
                            tq0 = ch * CH + ts * P
                            qq = tq0 // 256
                            r0 = 2 * ((tq0 % 256) // P)  # 0 or 2
                            for half in range(2):
                                r_ = r0 + half
                                src = slice(half * 64, (half + 1) * 64)
                                nc.sync.dma_start(
                                    zbs[qq][r_ * P: r_ * P + 64, :],
                                    z_re[src, trow, :])
                                nc.sync.dma_start(
                                    zbs[qq][r_ * P + 64: r_ * P + 128, :],
                                    z_im[src, trow, :])
                            # fire the sub-ReduceScatter once the quarter's
                            # second tile has landed
                            if ts % 2 == 1:
                                nc.gpsimd.collective_compute(
                                    "ReduceScatter", ALU.add,
                                    replica_groups=[[0, 1, 2, 3], [4, 5, 6, 7]],
                                    ins=[zbs[qq].opt()],
                                    outs=[rs_outs[qq].opt()],
                                )

            # ======== atan2(zim, zre)/pi, quarters batched in pairs ========
            for pp in range(2):
                zre_t = pools["nt"].tile([P, D], F32, tag="nt", name=f"zre{pp}")
                zim_t = pools["nt"].tile([P, D], F32, tag="nt", name=f"zim{pp}")
                qa, qb = 2 * pp, 2 * pp + 1
                nc.sync.dma_start(zre_t[0:64, :], rs_outs[qa][0:64, :])
                nc.sync.dma_start(zim_t[0:64, :], rs_outs[qa][64:128, :])
                nc.sync.dma_start(zre_t[64:128, :], rs_outs[qb][0:64, :])
                nc.sync.dma_start(zim_t[64:128, :], rs_outs[qb][64:128, :])
                nt = pools["nt"]

                def ft(nm, pp=pp):
                    return nt.tile([P, D], F32, tag="nt", name=f"{nm}{pp}")
                t1, t2, m, az = ft("f1"), ft("f2"), ft("f3"), ft("f5")
                den1, r1, ta0, ta = ft("f6"), ft("f7"), ft("f8"), ft("f9")
                num2, r2, tb0, tb = ft("fa"), ft("fb"), ft("fc"), ft("fd")
                ata, atb, mask = ft("fe"), ft("ff"), ft("fg")
                dsel, md, sel, outt = ft("fh"), ft("fi"), ft("fj"), ft("fk")
                for hw in range(2):
                    c = slice(hw * (D // 2), (hw + 1) * (D // 2))
                    zre, zim = zre_t[:, c], zim_t[:, c]
                    nc.scalar.activation(t1[:, c], zre, AF.Square,
                                         bias=0.0, scale=1.0)
                    nc.vector.tensor_tensor(t2[:, c], zim, zim, ALU.mult)
                    nc.vector.tensor_tensor(m[:, c], t1[:, c], t2[:, c],
                                            ALU.add)
                    nc.scalar.activation(az[:, c], m[:, c], AF.Sqrt,
                                         bias=0.0, scale=1.0)
                    nc.vector.tensor_tensor(den1[:, c], az[:, c], zre, ALU.add)
                    nc.vector.reciprocal_approx_fast(r1[:, c], den1[:, c])
                    nc.vector.tensor_tensor(ta0[:, c], zim, r1[:, c], ALU.mult)
                    nc.vector.tensor_scalar(ta[:, c], ta0[:, c], 1e8, -1e8,
                                            ALU.min, ALU.max)
                    nc.vector.tensor_tensor(num2[:, c], az[:, c], zre,
                                            ALU.subtract)
                    nc.vector.reciprocal_approx_fast(r2[:, c], zim)
                    nc.vector.tensor_tensor(tb0[:, c], num2[:, c], r2[:, c],
                                            ALU.mult)
                    nc.vector.tensor_scalar(tb[:, c], tb0[:, c], 1e8, -1e8,
                                            ALU.min, ALU.max)
                    nc.scalar.activation(ata[:, c], ta[:, c], AF.Arctan,
                                         bias=0.0, scale=1.0)
                    nc.scalar.activation(atb[:, c], tb[:, c], AF.Arctan,
                                         bias=0.0, scale=1.0)
                    nc.vector.tensor_scalar(mask[:, c], zre, 0.0, None,
                                            ALU.is_ge)
                    nc.vector.tensor_tensor(dsel[:, c], ata[:, c], atb[:, c],
                                            ALU.subtract)
                    nc.vector.tensor_tensor(md[:, c], mask[:, c], dsel[:, c],
                                            ALU.mult)
                    nc.vector.tensor_tensor(sel[:, c], atb[:, c], md[:, c],
                                            ALU.add)
                    nc.vector.tensor_scalar(outt[:, c], sel[:, c], 2.0 / PI,
                                            None, ALU.mult)
                nc.sync.dma_start(OUT[pp * P:(pp + 1) * P, :], outt[:, :])

    nc.finalize()
    return nc


_NC_CACHE = {}


def _get_nc():
    if "nc" not in _NC_CACHE:
        _NC_CACHE["nc"] = build()
    return _NC_CACHE["nc"]


def kernel(**inputs):
    query = np.ascontiguousarray(np.asarray(inputs["query"], dtype=np.float32))
    keyvalue = np.ascontiguousarray(np.asarray(inputs["keyvalue"], dtype=np.float32))
    wq = np.asarray(inputs["wq"], dtype=np.float32)
    wk = np.asarray(inputs["wk"], dtype=np.float32)
    wv = np.asarray(inputs["wv"], dtype=np.float32)
    bq = np.asarray(inputs["bq"], dtype=np.float32)
    bk = np.asarray(inputs["bk"], dtype=np.float32)
    bv = np.asarray(inputs["bv"], dtype=np.float32)
    wo = np.asarray(inputs["wo"], dtype=np.float32)
    bo = np.asarray(inputs["bo"], dtype=np.float32)

    in_maps = []
    for c in range(N_CORES):
        b, g = c // 4, c % 4
        h0 = g * HPC
        in_maps.append({
            "query": query[b],
            "keyvalue": keyvalue[b],
            "wq": np.ascontiguousarray(wq[h0:h0 + HPC]),
            "wk": np.ascontiguousarray(wk[h0:h0 + HPC]),
            "wv": np.ascontiguousarray(wv[h0:h0 + HPC]),
            "bq": np.ascontiguousarray(bq[h0:h0 + HPC]),
            "bk": np.ascontiguousarray(bk[h0:h0 + HPC]),
            "bv": np.ascontiguousarray(bv[h0:h0 + HPC]),
            "wo": np.ascontiguousarray(wo[h0 * D:(h0 + HPC) * D]),
            "bo": bo,
        })

    nc = _get_nc()
    res = run_bass_kernel_spmd(nc, in_maps, core_ids=list(range(N_CORES)))
    _NC_CACHE["last_results"] = res
    out = np.empty((B, T, D), np.float32)
    for c in range(N_CORES):
        b, g = c // 4, c % 4
        o = res.results[c]["out"]          # [256, 512]: 4 quarters x 64 rows
        for qq in range(4):
            out[b, qq * 256 + g * 64: qq * 256 + (g + 1) * 64, :] = \
                o[qq * 64:(qq + 1) * 64, :]
    return out
